# revision 1
# baseline (speedup 1.0000x reference)
"""GAT (single-head, PyG GATConv-style) message passing on 8 Trainium2 cores.

Strategy (per sharding hint): nodes partitioned by destination across 8
cores; each core owns a contiguous range of dst nodes and processes all
edges whose dst it owns.  The small weight matrices are replicated.

Math (equivalent to the reference, without max-subtraction in softmax):
    a_src = x @ (W_src @ att_src);  a_dst = x @ (W_dst @ att_dst)
    w_e   = exp(leaky_relu(a_src[src_e] + a_dst[dst_e]))
    U[i]  = sum_{e: dst=i} w_e * x[src_e]      (+ denom[i] = sum w_e)
    agg[i]= (U[i] @ W_src) / (denom[i] + EPS)   <- linearity of W_src
    out   = relu(agg + bias_conv) @ W_lin + b_lin

Device mapping:
  phase 1a: a_src[n] for all nodes written into column 128 of the
    augmented gather table xaug[N, 192] = [x | a_src | 1 | 0-pad]
    (computed as v_s^T @ x^T from a host-transposed copy of x).
  phase 1b: a_dst for this core's own nodes into ad_tab[12544, 64] col 0.
  phase 2: per 128-edge tile, dma_gather (GPSIMD custom op) fetches
    xaug[src] rows (768B) and ad_tab[dst_local] rows (256B); a weighted
    one-hot P_w^T = (iota==dst_local)*w is built on DVE and the tensor
    engine accumulates U[block] += P_w^T.T @ G in PSUM (the gathered
    "1" column makes the softmax denominator fall out of the same matmul).
  epilogue per 128-dst block: V = U/(denom+eps); aggT = W_src^T V^T (PE
    transpose + matmul); hT = relu(aggT + bias) on ACT (bias per-partition);
    out = hT^T @ W_lin + b_lin; DMA to the owned output rows.

dma_gather constraints honored: int16 indices (xaug gathered in 4 chunks
of 25088 rows, sections padded per (superblock, chunk, block) cell to a
cross-core-uniform tile count so one SPMD program serves all 8 cores),
elem size a multiple of 256B, indices wrapped [16, n/16] and replicated
across the 128 partitions.
"""

import os
import sys

sys.path.insert(0, "/opt/trn_rl_repo")

import numpy as np

# ----------------------------------------------------------------------------
# Problem constants (hardcoded per spec)
# ----------------------------------------------------------------------------
N_NODES = 100000
DIM = 128      # input dim D
HID = 128      # hidden dim H (= D here)
OUT = 64       # output dim O
N_CORES = 8
NEG_SLOPE = 0.2
EPS = 1e-16

P = 128          # partitions / edges per tile
RW = 192         # xaug row width (f32): 768B = [x(128) | a_src | 1 | 49 pad]
ADW = 64         # ad_tab row width (f32): 256B = [a_dst | 63 pad]
CHUNK = 25088    # xaug gather chunk rows (int16-indexable, 128-aligned)
SB_BLOCKS = 3    # dst blocks per superblock (PSUM + SBUF budget)
PHASE1_W = 512   # nodes per phase-1 matmul


def _ceil_div(a, b):
    return -(-a // b)


# ----------------------------------------------------------------------------
# Host-side sharding / routing prep (index manipulation only, no math on x)
# ----------------------------------------------------------------------------
def _prep_edges(edge_index, n_nodes, n_cores):
    src = np.asarray(edge_index[0], dtype=np.int64)
    dst = np.asarray(edge_index[1], dtype=np.int64)
    loops = np.arange(n_nodes, dtype=np.int64)
    src = np.concatenate([src, loops])
    dst = np.concatenate([dst, loops])

    npc = n_nodes // n_cores              # nodes per core (12500)
    nb = _ceil_div(npc, P)                # dst blocks per core (98)
    na = _ceil_div(n_nodes, PHASE1_W) * PHASE1_W   # padded phase-1a cols
    npc_pad = nb * P                      # ad_tab rows (12544)
    na_own = _ceil_div(npc_pad, PHASE1_W) * PHASE1_W
    nch = _ceil_div(n_nodes, CHUNK)       # xaug chunks (4)
    nsb = _ceil_div(nb, SB_BLOCKS)        # superblocks (33)

    per_core = []
    for c in range(n_cores):
        m = (dst >= c * npc) & (dst < (c + 1) * npc)
        s_c = src[m]
        d_c = dst[m] - c * npc
        blk = d_c >> 7
        ch = s_c // CHUNK
        order = np.lexsort((blk, ch, blk // SB_BLOCKS))
        per_core.append((s_c[order], d_c[order], blk[order], ch[order]))

    # per-(superblock, chunk, block) tile counts, uniform across cores
    cell_tiles = np.zeros((nsb, nch, nb), dtype=np.int64)
    for c in range(n_cores):
        s_c, d_c, blk, ch = per_core[c]
        cnt = np.zeros((nch, nb), dtype=np.int64)
        np.add.at(cnt, (ch, blk), 1)
        for b in range(nb):
            sb = b // SB_BLOCKS
            for k in range(nch):
                cell_tiles[sb, k, b] = max(cell_tiles[sb, k, b],
                                           _ceil_div(int(cnt[k, b]), P))

    # schedule: per sb, ordered (chunk, block) cells
    sched_sb = []        # per sb: dict with tile->block map, chunk tile counts
    t_global = 0
    for sb in range(nsb):
        blocks = list(range(sb * SB_BLOCKS, min((sb + 1) * SB_BLOCKS, nb)))
        tile_blocks = []
        chunk_nt = []
        for k in range(nch):
            nt_k = 0
            for b in blocks:
                tcnt = int(cell_tiles[sb, k, b])
                tile_blocks.extend([b] * tcnt)
                nt_k += tcnt
            chunk_nt.append(nt_k)
        T_sb = len(tile_blocks)
        # first/last tile (within sb) per block
        first = {}
        last = {}
        for i, b in enumerate(tile_blocks):
            if b not in first:
                first[b] = i
            last[b] = i
        sched_sb.append(dict(blocks=blocks, tile_blocks=tile_blocks,
                             chunk_nt=chunk_nt, first=first, last=last,
                             T=T_sb, t0=t_global))
        t_global += T_sb
    T_total = t_global

    def wrap16(flat):
        # [n] int -> [128, n//16] int16 wrapped + replicated
        n = len(flat)
        w = np.asarray(flat, np.int64).reshape(n // 16, 16).T
        assert w.min() >= 0 and w.max() < 32768
        return np.tile(w, (8, 1)).astype(np.int16)

    cores = []
    for c in range(n_cores):
        s_c, d_c, blk, ch = per_core[c]
        cnts = np.zeros((nsb, nch, nb), dtype=np.int64)
        np.add.at(cnts, (blk // SB_BLOCKS, ch, blk), 1)
        xidx_cols = []
        adidx_cols = []
        dloc_all = np.full((T_total, P), 999.0, dtype=np.float32)
        pos = 0  # cursor into the (sb, chunk, block)-sorted edge arrays
        t_cursor = 0
        for sb in range(nsb):
            sbs = sched_sb[sb]
            ad_segs = []
            for k in range(nch):
                if sbs["chunk_nt"][k] == 0:
                    continue
                x_segs = []
                for b in sbs["blocks"]:
                    n_e = int(cnts[sb, k, b])
                    tcnt = int(cell_tiles[sb, k, b])
                    if tcnt == 0:
                        assert n_e == 0
                        continue
                    e_s = s_c[pos : pos + n_e]
                    e_d = d_c[pos : pos + n_e]
                    pos += n_e
                    cell_x = np.zeros(tcnt * P, dtype=np.int64)
                    cell_ad = np.zeros(tcnt * P, dtype=np.int64)
                    cell_dl = np.full(tcnt * P, 999.0, dtype=np.float32)
                    cell_x[:n_e] = e_s - k * CHUNK
                    cell_ad[:n_e] = e_d
                    cell_dl[:n_e] = (e_d & 127).astype(np.float32)
                    dloc_all[t_cursor : t_cursor + tcnt] = cell_dl.reshape(
                        tcnt, P
                    )
                    x_segs.append(cell_x)
                    ad_segs.append(cell_ad)
                    t_cursor += tcnt
                xidx_cols.append(wrap16(np.concatenate(x_segs)))
            adidx_cols.append(wrap16(np.concatenate(ad_segs)))
        assert pos == len(s_c) and t_cursor == T_total
        xidx = np.concatenate(xidx_cols, axis=1) if xidx_cols else \
            np.zeros((128, 0), np.int16)
        adidx = np.concatenate(adidx_cols, axis=1)
        dloc = np.ascontiguousarray(dloc_all.T)  # [128, T_total]
        cores.append(dict(xidx=xidx, adidx=adidx, dloc=dloc))

    sched = dict(nsb=nsb, nch=nch, nb=nb, npc=npc, npc_pad=npc_pad, na=na,
                 na_own=na_own, T_total=T_total, sched_sb=sched_sb,
                 xcols=cores[0]["xidx"].shape[1],
                 adcols=cores[0]["adidx"].shape[1])
    return cores, sched


# ----------------------------------------------------------------------------
# Device program
# ----------------------------------------------------------------------------
def _build_program(sched, n_iters=1):
    import concourse.bass as bass
    import concourse.mybir as mybir
    import concourse.tile as tile
    from concourse import bacc
    from concourse import library_config

    f32 = mybir.dt.float32
    i16 = mybir.dt.int16
    Alu = mybir.AluOpType
    Act = mybir.ActivationFunctionType

    nsb, nch, nb, npc, npc_pad, na, na_own, T_total = (
        sched["nsb"], sched["nch"], sched["nb"], sched["npc"],
        sched["npc_pad"], sched["na"], sched["na_own"], sched["T_total"]
    )
    sched_sb = sched["sched_sb"]
    n1a = na // PHASE1_W
    n1b = na_own // PHASE1_W
    T_sb_max = max(s["T"] for s in sched_sb)

    nc = bacc.Bacc("TRN2", target_bir_lowering=False)

    xaug = nc.dram_tensor("xaug", [N_NODES, RW], f32, kind="ExternalInput")
    xT = nc.dram_tensor("xT", [P, na], f32, kind="ExternalInput")
    xT_own = nc.dram_tensor("xT_own", [P, na_own], f32, kind="ExternalInput")
    w_src = nc.dram_tensor("w_src", [DIM, HID], f32, kind="ExternalInput")
    w_dst = nc.dram_tensor("w_dst", [DIM, HID], f32, kind="ExternalInput")
    att2 = nc.dram_tensor("att2", [HID, 2], f32, kind="ExternalInput")
    w_lin = nc.dram_tensor("w_lin", [HID, OUT], f32, kind="ExternalInput")
    bias_c = nc.dram_tensor("bias_c", [HID, 1], f32, kind="ExternalInput")
    blin_b = nc.dram_tensor("blin_b", [P, OUT], f32, kind="ExternalInput")
    iota_in = nc.dram_tensor("iota", [P, P], f32, kind="ExternalInput")
    ident_in = nc.dram_tensor("ident", [P, P], f32, kind="ExternalInput")
    xidx_in = nc.dram_tensor("xidx", [P, max(sched["xcols"], 8)], i16,
                             kind="ExternalInput")
    adidx_in = nc.dram_tensor("adidx", [P, sched["adcols"]], i16,
                              kind="ExternalInput")
    dloc_in = nc.dram_tensor("dloc", [P, T_total], f32, kind="ExternalInput")
    ad_tab = nc.dram_tensor("ad_tab", [npc_pad, ADW], f32,
                            kind="ExternalInput")  # zeros; col 0 written here

    y_out = nc.dram_tensor("y", [npc, OUT], f32, kind="ExternalOutput")

    with tile.TileContext(nc) as tc:
        nc.gpsimd.load_library(library_config.mlp)
        with (
            tc.tile_pool(name="const", bufs=1) as cpool,
            tc.tile_pool(name="p1x", bufs=3) as p1pool,
            tc.tile_pool(name="p1a", bufs=3) as p1apool,
            tc.tile_pool(name="gx", bufs=2) as gxpool,
            tc.tile_pool(name="adg", bufs=2) as adpool,
            tc.tile_pool(name="idx", bufs=2) as ipool,
            tc.tile_pool(name="small", bufs=3) as spool,
            tc.tile_pool(name="pwt", bufs=4) as pwtpool,
            tc.tile_pool(name="epi", bufs=3) as epool,
            tc.tile_pool(name="ps_misc", bufs=3, space="PSUM") as psm,
            tc.tile_pool(name="ps_u", bufs=4, space="PSUM") as psu,
            tc.tile_pool(name="ps_p1", bufs=1, space="PSUM") as psp1,
        ):
            # ---- constants
            iota_sb = cpool.tile([P, P], f32)
            ident_sb = cpool.tile([P, P], f32)
            wsrc_sb = cpool.tile([DIM, HID], f32)
            wdst_sb = cpool.tile([DIM, HID], f32)
            att2_sb = cpool.tile([HID, 2], f32)
            wlin_sb = cpool.tile([HID, OUT], f32)
            biasc_sb = cpool.tile([HID, 1], f32)
            blinb_sb = cpool.tile([P, OUT], f32)
            for sb_t, dr in (
                (iota_sb, iota_in), (ident_sb, ident_in), (wsrc_sb, w_src),
                (wdst_sb, w_dst), (att2_sb, att2), (wlin_sb, w_lin),
                (biasc_sb, bias_c), (blinb_sb, blin_b),
            ):
                nc.sync.dma_start(out=sb_t[:], in_=dr[:])

            # ---- phase 0: v2 = [W_src @ att_src | W_dst @ att_dst]
            v2_sb = cpool.tile([DIM, 2], f32)
            for k, wmat in enumerate((wsrc_sb, wdst_sb)):
                wt_ps = psm.tile([P, P], f32, tag="mm")
                nc.tensor.transpose(wt_ps[:], wmat[:], ident_sb[:])
                wt_sb = epool.tile([P, P], f32, tag="wt")
                nc.vector.tensor_copy(out=wt_sb[:], in_=wt_ps[:])
                v_ps = psm.tile([DIM, 1], f32, tag="mm")
                nc.tensor.matmul(v_ps[:], wt_sb[:], att2_sb[:, k : k + 1],
                                 start=True, stop=True)
                nc.vector.tensor_copy(out=v2_sb[:, k : k + 1], in_=v_ps[:])

            def body(_it=None):
                # ---- phase 1a: a_src -> xaug[:, 128]
                for i in range(n1a):
                    xt_t = p1pool.tile([P, PHASE1_W], f32, tag="xt")
                    nc.sync.dma_start(
                        out=xt_t[:], in_=xT[:, i * PHASE1_W : (i + 1) * PHASE1_W]
                    )
                    a_ps = psp1.tile([1, PHASE1_W], f32, tag="a")
                    nc.tensor.matmul(a_ps[:], v2_sb[:, 0:1], xt_t[:],
                                     start=True, stop=True)
                    a_sb = p1apool.tile([1, PHASE1_W], f32, tag="as")
                    nc.scalar.activation(a_sb[:], a_ps[:], Act.Copy)
                    r0 = i * PHASE1_W
                    r1 = min(r0 + PHASE1_W, N_NODES)
                    if r1 > r0:
                        nc.sync.dma_start(
                            out=xaug[r0:r1, 128:129], in_=a_sb[:1, : r1 - r0]
                        )
                # ---- phase 1b: own a_dst -> ad_tab[:, 0]
                for i in range(n1b):
                    xt_t = p1pool.tile([P, PHASE1_W], f32, tag="xt")
                    nc.sync.dma_start(
                        out=xt_t[:],
                        in_=xT_own[:, i * PHASE1_W : (i + 1) * PHASE1_W],
                    )
                    a_ps = psp1.tile([1, PHASE1_W], f32, tag="a")
                    nc.tensor.matmul(a_ps[:], v2_sb[:, 1:2], xt_t[:],
                                     start=True, stop=True)
                    a_sb = p1apool.tile([1, PHASE1_W], f32, tag="as")
                    nc.scalar.activation(a_sb[:], a_ps[:], Act.Copy)
                    r0 = i * PHASE1_W
                    r1 = min(r0 + PHASE1_W, npc_pad)
                    if r1 > r0:
                        nc.sync.dma_start(
                            out=ad_tab[r0:r1, 0:1], in_=a_sb[:1, : r1 - r0]
                        )

                # ---- phase 2
                xcol = 0
                adcol = 0
                u_ps = {}
                for sb in range(nsb):
                    sbs = sched_sb[sb]
                    T_sb = sbs["T"]
                    if T_sb == 0:
                        continue
                    gx_t = gxpool.tile([P, T_sb_max * RW], f32)
                    adg_t = adpool.tile([P, T_sb_max * ADW], f32)
                    gx3 = gx_t[:].rearrange("p (t e) -> p t e", e=RW)
                    adg3 = adg_t[:].rearrange("p (t e) -> p t e", e=ADW)

                    # index tiles
                    nxcols = sum(sbs["chunk_nt"]) * 8
                    xidx_t = ipool.tile([P, max(T_sb_max * 8, 8)], i16, tag="xi")
                    nc.sync.dma_start(
                        out=xidx_t[:, :nxcols],
                        in_=xidx_in[:, xcol : xcol + nxcols],
                    )
                    adidx_t = ipool.tile([P, max(T_sb_max * 8, 8)], i16, tag="ai")
                    nc.sync.dma_start(
                        out=adidx_t[:, : T_sb * 8],
                        in_=adidx_in[:, adcol : adcol + T_sb * 8],
                    )
                    dloc_t = spool.tile([P, T_sb_max], f32, tag="dloc")
                    nc.sync.dma_start(
                        out=dloc_t[:, :T_sb],
                        in_=dloc_in[:, sbs["t0"] : sbs["t0"] + T_sb],
                    )

                    # gathers (SWDGE ring caps one dma_gather at 1024 idxs)
                    MAXT = 8
                    t_off = 0
                    ic_off = 0
                    for k in range(nch):
                        nt_k = sbs["chunk_nt"][k]
                        if nt_k == 0:
                            continue
                        c0 = k * CHUNK
                        c1 = min(c0 + CHUNK, N_NODES)
                        for s0 in range(0, nt_k, MAXT):
                            sn = min(MAXT, nt_k - s0)
                            nc.gpsimd.dma_gather(
                                out_ap=gx3[:, t_off + s0 : t_off + s0 + sn, :],
                                in_ap=xaug[c0:c1, :],
                                idxs_ap=xidx_t[
                                    :, ic_off + s0 * 8 : ic_off + (s0 + sn) * 8
                                ],
                                num_idxs=sn * P,
                                num_idxs_reg=sn * P,
                                elem_size=RW,
                            )
                        t_off += nt_k
                        ic_off += nt_k * 8
                    xcol += nxcols
                    for s0 in range(0, T_sb, MAXT):
                        sn = min(MAXT, T_sb - s0)
                        nc.gpsimd.dma_gather(
                            out_ap=adg3[:, s0 : s0 + sn, :],
                            in_ap=ad_tab[:, :],
                            idxs_ap=adidx_t[:, s0 * 8 : (s0 + sn) * 8],
                            num_idxs=sn * P,
                            num_idxs_reg=sn * P,
                            elem_size=ADW,
                        )
                    adcol += T_sb * 8

                    # w = exp(leaky_relu(a_src + a_dst)) per tile column
                    z_t = spool.tile([P, T_sb_max], f32, tag="z")
                    z2_t = spool.tile([P, T_sb_max], f32, tag="z2")
                    w_t = spool.tile([P, T_sb_max], f32, tag="w")
                    nc.vector.tensor_tensor(
                        out=z_t[:, :T_sb], in0=gx3[:, :T_sb, 128],
                        in1=adg3[:, :T_sb, 0], op=Alu.add,
                    )
                    nc.vector.tensor_scalar(
                        out=z2_t[:, :T_sb], in0=z_t[:, :T_sb],
                        scalar1=NEG_SLOPE, scalar2=None, op0=Alu.mult,
                    )
                    nc.vector.tensor_tensor(
                        out=z2_t[:, :T_sb], in0=z_t[:, :T_sb],
                        in1=z2_t[:, :T_sb], op=Alu.max,
                    )
                    nc.scalar.activation(w_t[:, :T_sb], z2_t[:, :T_sb], Act.Exp)

                    # matmuls per tile
                    for t in range(T_sb):
                        b = sbs["tile_blocks"][t]
                        if sbs["first"][b] == t:
                            u_ps[b] = psu.tile([P, 130], f32, tag="u", name="u_ps")
                        pwt_t = pwtpool.tile([P, P], f32)
                        nc.vector.tensor_scalar(
                            out=pwt_t[:], in0=iota_sb[:],
                            scalar1=dloc_t[:, t : t + 1],
                            scalar2=w_t[:, t : t + 1],
                            op0=Alu.is_equal, op1=Alu.mult,
                        )
                        nc.tensor.matmul(
                            u_ps[b][:], pwt_t[:],
                            gx_t[:, t * RW : t * RW + 130],
                            start=(sbs["first"][b] == t),
                            stop=(sbs["last"][b] == t),
                        )
                        if sbs["last"][b] == t:
                            ub = u_ps.pop(b)
                            # ---- epilogue for block b
                            rcp_t = epool.tile([P, 1], f32, tag="rcp")
                            nc.vector.tensor_scalar(
                                out=rcp_t[:], in0=ub[:, 129:130],
                                scalar1=float(EPS), scalar2=None, op0=Alu.add,
                            )
                            nc.vector.reciprocal(rcp_t[:], rcp_t[:])
                            v_t = epool.tile([P, DIM], f32, tag="v")
                            nc.vector.tensor_scalar(
                                out=v_t[:], in0=ub[:, :DIM],
                                scalar1=rcp_t[:, :1], scalar2=None,
                                op0=Alu.mult,
                            )
                            vt_ps = psm.tile([P, P], f32, tag="mm")
                            nc.tensor.transpose(vt_ps[:], v_t[:], ident_sb[:])
                            vt_sb = epool.tile([DIM, P], f32, tag="vt")
                            nc.vector.tensor_copy(out=vt_sb[:], in_=vt_ps[:])
                            aggt_ps = psm.tile([HID, P], f32, tag="mm")
                            nc.tensor.matmul(aggt_ps[:], wsrc_sb[:], vt_sb[:],
                                             start=True, stop=True)
                            ht_sb = epool.tile([HID, P], f32, tag="ht")
                            nc.scalar.activation(
                                ht_sb[:], aggt_ps[:], Act.Relu,
                                bias=biasc_sb[:, :1],
                            )
                            o_ps = psm.tile([P, OUT], f32, tag="mm")
                            nc.tensor.matmul(o_ps[:], ht_sb[:], wlin_sb[:],
                                             start=True, stop=True)
                            o_sb = epool.tile([P, OUT], f32, tag="osb")
                            nc.vector.tensor_tensor(
                                out=o_sb[:], in0=o_ps[:], in1=blinb_sb[:],
                                op=Alu.add,
                            )
                            r0 = b * P
                            r1 = min(r0 + P, npc)
                            nc.sync.dma_start(
                                out=y_out[r0:r1, :], in_=o_sb[: r1 - r0, :]
                            )

            if n_iters == 1:
                body()
            else:
                with tc.For_i(0, n_iters, 1) as it:
                    body(it)

    nc.compile()
    return nc


# ----------------------------------------------------------------------------
# Host input assembly
# ----------------------------------------------------------------------------
def _make_in_maps(x, W_src, W_dst, att_src, att_dst, bias_conv, W_lin, b_lin,
                  cores, sched):
    na, na_own, npc, npc_pad = (sched["na"], sched["na_own"], sched["npc"],
                                sched["npc_pad"])
    x = np.asarray(x, dtype=np.float32)
    n = x.shape[0]
    xaug = np.zeros((n, RW), np.float32)
    xaug[:, :DIM] = x
    xaug[:, 129] = 1.0          # col 128 = a_src (device-filled)
    xT = np.zeros((P, na), np.float32)
    xT[:, :n] = x.T
    att2 = np.stack([np.asarray(att_src, np.float32),
                     np.asarray(att_dst, np.float32)], axis=1)
    bias_c = np.asarray(bias_conv, np.float32).reshape(HID, 1).copy()
    blin_b = np.broadcast_to(np.asarray(b_lin, np.float32), (P, OUT)).copy()
    iota = np.broadcast_to(np.arange(P, dtype=np.float32), (P, P)).copy()
    ident = np.eye(P, dtype=np.float32)
    ad_tab = np.zeros((npc_pad, ADW), np.float32)

    common = dict(
        xaug=xaug, xT=xT,
        w_src=np.asarray(W_src, np.float32), w_dst=np.asarray(W_dst, np.float32),
        att2=att2, w_lin=np.asarray(W_lin, np.float32),
        bias_c=bias_c, blin_b=blin_b, iota=iota, ident=ident, ad_tab=ad_tab,
    )
    in_maps = []
    for c in range(len(cores)):
        m = dict(common)
        xT_own = np.zeros((P, na_own), np.float32)
        r1 = min((c + 1) * npc, n)
        xT_own[:, : r1 - c * npc] = x[c * npc : r1].T
        m["xT_own"] = xT_own
        xi = cores[c]["xidx"]
        if xi.shape[1] < 8:
            xi = np.zeros((P, 8), np.int16)
        m["xidx"] = xi
        m["adidx"] = cores[c]["adidx"]
        m["dloc"] = cores[c]["dloc"]
        in_maps.append(m)
    return in_maps


# ----------------------------------------------------------------------------
# Public entry point
# ----------------------------------------------------------------------------
def kernel(x, W_src, W_dst, att_src, att_dst, bias_conv, W_lin, b_lin,
           edge_index):
    from concourse.bass_utils import run_bass_kernel_spmd

    cores, sched = _prep_edges(edge_index, N_NODES, N_CORES)
    n_iters = int(os.environ.get("GAT_KERNEL_ITERS", "1"))
    nc = _build_program(sched, n_iters=n_iters)
    in_maps = _make_in_maps(x, W_src, W_dst, att_src, att_dst, bias_conv,
                            W_lin, b_lin, cores, sched)
    res = run_bass_kernel_spmd(nc, in_maps, core_ids=list(range(N_CORES)))
    y = np.concatenate([res.results[c]["y"] for c in range(N_CORES)], axis=0)
    return np.ascontiguousarray(y[:N_NODES]).astype(np.float32)

